# revision 4
# baseline (speedup 1.0000x reference)
"""Bidirectional Echo State Network (BiESN) on 8 Trainium2 NeuronCores.

Strategy
--------
The ESN recurrence  h_t = 0.1*h_{t-1} + 0.9*tanh(x_t@K + h_{t-1}@R)  is strictly
serial over T but independent per (direction, sample). The only wall-clock
parallelism for the recurrence is the 2 directions; batch splits freely.

Sharding: 8 cores = 2 directions x 4 batch-groups of B/4 samples. Every core
runs the *same* program (pure SPMD): a forward ESN over its inputs. The host
feeds the backward cores a time-reversed x and un-reverses their outputs.

Per-core device program (all layouts transposed: state kept as [U, batch] so
each step's matmul output feeds the next step's moving operand directly, with
rec tiles as the PE stationary operand -> zero on-device transposes):
  1. GEMM: xinT[u, (b,t)] = sum_d K[d,u] * xT[d,(b,t)] + bias[u]  (PSUM f32,
     evacuated by ACT with the per-partition bias add, fp16 to DRAM scratch).
  2. Recurrence, 2048 steps: per step 16 matmuls (4 k-tiles x 4 m-tiles of R,
     stationary fp16 [128,128] tiles -> FWL weight loads), PSUM += R[k,m]^T @
     h_k; then DVE adds xin, ACT tanh, DVE blend -> new state tiles, which are
     also the output columns.
All host work (slicing, transposes, dtype casts, concat) is numpy plumbing.

fp16 everywhere on-device (f32 accumulation in PSUM / DVE / ACT): validated
~3e-4 relative error vs the f32 reference (error feedback is contractive:
spectral radius 0.95, leak 0.9, tanh saturation).
"""

import sys
import types

for _p in ("/opt/trn_rl_repo", "/root/.axon_site"):
    if _p not in sys.path:
        sys.path.insert(0, _p)

import numpy as np

LEAKY = 0.9
N_CORES = 8
P = 128  # partitions


def _split_multi_waits(nc):
    """This walrus build accepts at most ONE sync-wait per instruction
    (setupSyncWait: "Too many sync wait commands"). Engines execute their
    streams in order, so hoisting the extra waits onto single-wait NOPs
    immediately before the instruction is semantically identical."""
    import concourse.mybir as mybir

    ctr = 0
    for f in nc.m.functions:
        for b in f.blocks:
            insts = b.instructions
            if not any(
                i.sync_info and i.sync_info.on_wait and len(i.sync_info.on_wait) > 1
                for i in insts
            ):
                continue
            out = []
            for inst in insts:
                si = inst.sync_info
                waits = list(si.on_wait) if si and si.on_wait else []
                if len(waits) > 1:
                    for w in waits[:-1]:
                        ctr += 1
                        nop = mybir.InstNoOp(name=f"waitsplit-{ctr}")
                        nop.engine = inst.engine
                        nop.sync_info = mybir.SyncInfo(on_wait=[w], on_update=[])
                        out.append(nop)
                    inst.sync_info = mybir.SyncInfo(
                        on_wait=[waits[-1]],
                        on_update=list(si.on_update) if si.on_update else [],
                    )
                out.append(inst)
            b.instructions = out


def _apply_ntff_shim():
    """Register the axon NTFF profile hook (missing antenv.axon_hooks) so
    run_bass_kernel_spmd(trace=True) can report HW exec time."""
    if "antenv.axon_hooks" in sys.modules:
        return
    try:
        from trn_agent_boot.trn_boot import _ntff_profile_via_ctypes

        hook = _ntff_profile_via_ctypes("/opt/axon/libaxon_pjrt.so")
    except Exception:
        hook = None
    mod = types.ModuleType("antenv.axon_hooks")
    mod.get_axon_ntff_profile_hook = lambda: hook
    mod.set_axon_ntff_profile_hook = lambda h: None
    sys.modules["antenv.axon_hooks"] = mod


def _build_program(T, B_c, D, U, T_blk=128):
    """Build the per-core SPMD Bass program. Returns the Bass object."""
    import concourse.bass as bass
    import concourse.mybir as mybir
    import concourse.tile as tile

    dt = mybir.dt
    DT = dt.float16
    F32 = dt.float32
    AF = mybir.ActivationFunctionType
    Alu = mybir.AluOpType

    KD = D // P  # k-tiles over input dim
    KU = U // P  # k/m-tiles over reservoir dim (4)
    assert U == 512 and D % P == 0 and T % T_blk == 0
    NCOL = B_c * T  # GEMM moving columns
    GN = 512  # GEMM N-block
    assert NCOL % GN == 0

    nc = bass.Bass(
        "TRN2", target_bir_lowering=False, debug=False, num_devices=N_CORES
    )
    xT_d = nc.dram_tensor("xt", [D, NCOL], DT, kind="ExternalInput")
    wk_d = nc.dram_tensor("wk", [D, U], DT, kind="ExternalInput")
    wr_d = nc.dram_tensor("wr", [U, U], DT, kind="ExternalInput")
    bias_d = nc.dram_tensor("bias", [KU, P, 1], F32, kind="ExternalInput")
    out_d = nc.dram_tensor("out", [U, B_c, T], DT, kind="ExternalOutput")

    with tile.TileContext(nc) as tc:
        with (
            tc.tile_pool(name="consts", bufs=1) as cpool,
            tc.tile_pool(name="dram", bufs=1, space="DRAM") as dpool,
        ):
            # resident weights: wk_sb/wr_sb [128, KD*U] with col = k*U + u
            wk_sb = cpool.tile([P, KD * U], DT, tag="wk", name="wk")
            wr_sb = cpool.tile([P, KU * U], DT, tag="wr", name="wr")
            bias_sb = cpool.tile([P, KU], F32, tag="bias", name="bias")
            for k in range(KD):
                nc.sync.dma_start(
                    wk_sb[:, k * U : (k + 1) * U], wk_d[k * P : (k + 1) * P, :]
                )
            for k in range(KU):
                nc.sync.dma_start(
                    wr_sb[:, k * U : (k + 1) * U], wr_d[k * P : (k + 1) * P, :]
                )
            for m in range(KU):
                nc.sync.dma_start(bias_sb[:, m : m + 1], bias_d[m])

            xin_dram = dpool.tile([U, NCOL], DT, tag="xin", name="xin")

            # ---------------- Phase 1: input GEMM ----------------
            with (
                tc.tile_pool(name="g_xt", bufs=3) as xt_pool,
                tc.tile_pool(name="g_ps", bufs=8, space="PSUM") as gps_pool,
                tc.tile_pool(name="g_ev", bufs=3) as ev_pool,
            ):
                for j in range(NCOL // GN):
                    xts = []
                    for k in range(KD):
                        xt = xt_pool.tile([P, GN], DT, tag=f"xt{k}", name=f"xt{k}")
                        nc.sync.dma_start(
                            xt[:],
                            xT_d[k * P : (k + 1) * P, j * GN : (j + 1) * GN],
                        )
                        xts.append(xt)
                    for m in range(KU):
                        ps = gps_pool.tile([P, GN], F32, tag="gps", name="gps")
                        for k in range(KD):
                            nc.tensor.matmul(
                                ps[:],
                                wk_sb[:, k * U + m * P : k * U + (m + 1) * P],
                                xts[k][:],
                                start=(k == 0),
                                stop=(k == KD - 1),
                            )
                        ev = ev_pool.tile([P, GN], DT, tag="ev", name="ev")
                        nc.scalar.activation(
                            ev[:], ps[:], AF.Identity, bias=bias_sb[:, m : m + 1]
                        )
                        nc.sync.dma_start(
                            xin_dram[m * P : (m + 1) * P, j * GN : (j + 1) * GN],
                            ev[:],
                        )

            # ---------------- Phase 2: recurrence ----------------
            # groups g=0,1 each cover m-tiles (2g, 2g+1); within-group col
            # order n = 8*mi + b so psum/u/state/out column layouts agree.
            NB = 2 * B_c  # columns per group op
            with (
                tc.tile_pool(name="r_u", bufs=1) as u_pool,
                tc.tile_pool(name="r_o", bufs=1) as o_pool,
                tc.tile_pool(name="r_ps", bufs=2, space="PSUM") as rps_pool,
                tc.tile_pool(name="r_tmp", bufs=2) as tmp_pool,
                tc.tile_pool(name="r_state", bufs=1) as st_pool,
            ):
                h_state = [
                    st_pool.tile([P, NB], DT, tag=f"hs{g}", name=f"hs{g}") for g in range(2)
                ]
                for g in range(2):
                    nc.vector.memset(h_state[g][:], 0.0)

                xin_v = [
                    xin_dram[m * P : (m + 1) * P, :].rearrange(
                        "p (b t) -> p b t", t=T
                    )
                    for m in range(KU)
                ]
                out_v = [out_d[m * P : (m + 1) * P, :, :] for m in range(KU)]

                def body(i):
                    u_g = [
                        u_pool.tile([P, NB * T_blk], DT, tag=f"u{g}", name=f"u{g}")
                        for g in range(2)
                    ]
                    o_g = [
                        o_pool.tile([P, NB * T_blk], DT, tag=f"o{g}", name=f"o{g}")
                        for g in range(2)
                    ]
                    for g in range(2):
                        for mi in range(2):
                            m = 2 * g + mi
                            dst = u_g[g][
                                :, mi * B_c * T_blk : (mi + 1) * B_c * T_blk
                            ].rearrange("p (b t) -> p b t", t=T_blk)
                            nc.sync.dma_start(
                                dst, xin_v[m][:, :, bass.ds(i, T_blk)]
                            )

                    def state_ap(k, t):
                        # [128, B_c] moving operand for k-tile at step t
                        g, mi = k // 2, k % 2
                        if t == 0:
                            return h_state[g][:, mi * B_c : (mi + 1) * B_c]
                        return o_g[g].rearrange("p (n t) -> p n t", t=T_blk)[
                            :, mi * B_c : (mi + 1) * B_c, t - 1
                        ]

                    for t in range(T_blk):
                        for g in range(2):
                            ps = rps_pool.tile([P, NB], F32, tag=f"ps{g}", name=f"ps{g}")
                            for mi in range(2):
                                m = 2 * g + mi
                                for k in range(KU):
                                    nc.tensor.matmul(
                                        ps[:, mi * B_c : (mi + 1) * B_c],
                                        wr_sb[
                                            :,
                                            k * U + m * P : k * U + (m + 1) * P,
                                        ],
                                        state_ap(k, t),
                                        start=(k == 0),
                                        stop=(k == KU - 1),
                                    )
                            u_ap = u_g[g].rearrange(
                                "p (n t) -> p n t", t=T_blk
                            )[:, :, t]
                            if t == 0:
                                hp = h_state[g][:]
                            else:
                                hp = o_g[g].rearrange(
                                    "p (n t) -> p n t", t=T_blk
                                )[:, :, t - 1]
                            v = tmp_pool.tile([P, NB], F32, tag=f"v{g}", name=f"v{g}")
                            nc.vector.tensor_add(v[:], ps[:], u_ap)
                            th = tmp_pool.tile([P, NB], F32, tag=f"th{g}", name=f"th{g}")
                            nc.scalar.activation(th[:], v[:], AF.Tanh)
                            # h_new = hp + 0.9*(th - hp)
                            nc.vector.tensor_sub(v[:], th[:], hp)
                            o_ap = o_g[g].rearrange(
                                "p (n t) -> p n t", t=T_blk
                            )[:, :, t]
                            nc.vector.scalar_tensor_tensor(
                                o_ap, v[:], float(LEAKY), hp, Alu.mult, Alu.add
                            )

                    for g in range(2):
                        nc.vector.tensor_copy(
                            h_state[g][:],
                            o_g[g].rearrange("p (n t) -> p n t", t=T_blk)[
                                :, :, T_blk - 1
                            ],
                        )
                        for mi in range(2):
                            m = 2 * g + mi
                            src = u_g  # noqa (clarity)
                            nc.sync.dma_start(
                                out_v[m][:, :, bass.ds(i, T_blk)],
                                o_g[g][
                                    :, mi * B_c * T_blk : (mi + 1) * B_c * T_blk
                                ].rearrange("p (b t) -> p b t", t=T_blk),
                            )

                import concourse.mybir as _mybir

                with tc.For_i(
                    0,
                    T,
                    T_blk,
                    hint_engines=(
                        _mybir.EngineType.PE,
                        _mybir.EngineType.DVE,
                        _mybir.EngineType.Activation,
                    ),
                ) as i:
                    body(i)

    _split_multi_waits(nc)
    return nc


_PROG_CACHE = {}


def _get_program(T, B_c, D, U):
    key = (T, B_c, D, U)
    if key not in _PROG_CACHE:
        _apply_ntff_shim()
        _PROG_CACHE[key] = _build_program(T, B_c, D, U)
    return _PROG_CACHE[key]


def _run(x, kernel_f, rec_f, bias_f, kernel_b, rec_b, bias_b, trace=False):
    from concourse.bass_utils import run_bass_kernel_spmd

    x = np.asarray(x)
    B, T, D = x.shape
    U = np.asarray(kernel_f).shape[1]
    n_grp = N_CORES // 2
    assert B % n_grp == 0
    B_c = B // n_grp

    nc = _get_program(T, B_c, D, U)

    wks = [np.asarray(kernel_f), np.asarray(kernel_b)]
    wrs = [np.asarray(rec_f), np.asarray(rec_b)]
    biases = [np.asarray(bias_f), np.asarray(bias_b)]

    in_maps = []
    metas = []
    for c in range(N_CORES):
        d, g = c // n_grp, c % n_grp
        xs = x[g * B_c : (g + 1) * B_c]
        if d == 1:
            xs = xs[:, ::-1, :]
        xT = np.ascontiguousarray(
            xs.transpose(2, 0, 1).reshape(D, B_c * T)
        ).astype(np.float16)
        in_maps.append(
            {
                "xt": xT,
                "wk": wks[d].astype(np.float16),
                "wr": wrs[d].astype(np.float16),
                "bias": biases[d].reshape(U // P, P, 1).astype(np.float32),
            }
        )
        metas.append((d, g))

    res = run_bass_kernel_spmd(nc, in_maps, list(range(N_CORES)), trace=trace)

    out = np.empty((B, T, 2 * U), dtype=np.float32)
    for c in range(N_CORES):
        d, g = metas[c]
        o = res.results[c]["out"].astype(np.float32)  # [U, B_c, T]
        o = o.transpose(1, 2, 0)  # [B_c, T, U]
        if d == 1:
            o = o[:, ::-1, :]
        out[g * B_c : (g + 1) * B_c, :, d * U : (d + 1) * U] = o
    return out, res


def kernel(x, kernel_f, rec_f, bias_f, kernel_b, rec_b, bias_b):
    out, _ = _run(x, kernel_f, rec_f, bias_f, kernel_b, rec_b, bias_b)
    return out


# revision 9
# speedup vs baseline: 1.2708x; 1.2708x over previous
"""Bidirectional Echo State Network (BiESN) on 8 Trainium2 NeuronCores.

Strategy
--------
The ESN recurrence  h_t = 0.1*h_{t-1} + 0.9*tanh(x_t@K + h_{t-1}@R)  is strictly
serial over T but independent per (direction, sample). The only wall-clock
parallelism for the recurrence is the 2 directions; batch splits freely.

Sharding: 8 cores = 2 directions x 4 batch-groups of B/4 samples. Every core
runs the *same* program (pure SPMD): a forward ESN over its inputs. The host
feeds the backward cores a time-reversed x and un-reverses their outputs.

Per-core device program (all layouts transposed: state kept as [U, batch] so
each step's matmul output feeds the next step's moving operand directly, with
rec tiles as the PE stationary operand -> zero on-device transposes):
  1. GEMM: xinT[u, (b,t)] = sum_d K[d,u] * xT[d,(b,t)] + bias[u]  (PSUM f32,
     evacuated by ACT with the per-partition bias add, fp16 to DRAM scratch).
  2. Recurrence, 2048 steps: per step 16 matmuls (4 k-tiles x 4 m-tiles of R,
     stationary fp16 [128,128] tiles -> FWL weight loads), PSUM += R[k,m]^T @
     h_k; then DVE adds xin, ACT tanh, DVE blend -> new state tiles, which are
     also the output columns.
All host work (slicing, transposes, dtype casts, concat) is numpy plumbing.

fp16 everywhere on-device (f32 accumulation in PSUM / DVE / ACT): validated
~3e-4 relative error vs the f32 reference (error feedback is contractive:
spectral radius 0.95, leak 0.9, tanh saturation).
"""

import sys
import types

for _p in ("/opt/trn_rl_repo", "/root/.axon_site"):
    if _p not in sys.path:
        sys.path.insert(0, _p)

import numpy as np

LEAKY = 0.9
N_CORES = 8
P = 128  # partitions


def _split_multi_waits(nc):
    """This walrus build accepts at most ONE sync-wait per instruction
    (setupSyncWait: "Too many sync wait commands"). Engines execute their
    streams in order, so hoisting the extra waits onto single-wait NOPs
    immediately before the instruction is semantically identical."""
    import concourse.mybir as mybir

    ctr = 0
    for f in nc.m.functions:
        for b in f.blocks:
            insts = b.instructions
            if not any(
                i.sync_info and i.sync_info.on_wait and len(i.sync_info.on_wait) > 1
                for i in insts
            ):
                continue
            out = []
            for inst in insts:
                si = inst.sync_info
                waits = list(si.on_wait) if si and si.on_wait else []
                if len(waits) > 1:
                    for w in waits[:-1]:
                        ctr += 1
                        nop = mybir.InstNoOp(name=f"waitsplit-{ctr}")
                        nop.engine = inst.engine
                        nop.sync_info = mybir.SyncInfo(on_wait=[w], on_update=[])
                        out.append(nop)
                    inst.sync_info = mybir.SyncInfo(
                        on_wait=[waits[-1]],
                        on_update=list(si.on_update) if si.on_update else [],
                    )
                out.append(inst)
            b.instructions = out


def _apply_ntff_shim():
    """Register the axon NTFF profile hook (missing antenv.axon_hooks) so
    run_bass_kernel_spmd(trace=True) can report HW exec time."""
    if "antenv.axon_hooks" in sys.modules:
        return
    try:
        from trn_agent_boot.trn_boot import _ntff_profile_via_ctypes

        hook = _ntff_profile_via_ctypes("/opt/axon/libaxon_pjrt.so")
    except Exception:
        hook = None
    mod = types.ModuleType("antenv.axon_hooks")
    mod.get_axon_ntff_profile_hook = lambda: hook
    mod.set_axon_ntff_profile_hook = lambda h: None
    sys.modules["antenv.axon_hooks"] = mod


def _build_program(T, B_c, D, U, T_blk=128):
    """Build the per-core SPMD Bass program. Returns the Bass object."""
    import concourse.bass as bass
    import concourse.mybir as mybir
    import concourse.tile as tile

    dt = mybir.dt
    DT = dt.float16
    F32 = dt.float32
    AF = mybir.ActivationFunctionType
    Alu = mybir.AluOpType

    KD = D // P  # k-tiles over input dim
    KU = U // P  # k/m-tiles over reservoir dim (4)
    assert U == 512 and D % P == 0 and T % T_blk == 0
    NCOL = B_c * T  # GEMM moving columns
    GN = 512  # GEMM N-block
    assert NCOL % GN == 0

    nc = bass.Bass(
        "TRN2", target_bir_lowering=False, debug=False, num_devices=N_CORES
    )
    xT_d = nc.dram_tensor("xt", [D, NCOL], DT, kind="ExternalInput")
    wk_d = nc.dram_tensor("wk", [D, U], DT, kind="ExternalInput")
    wr_d = nc.dram_tensor("wr", [U, U], DT, kind="ExternalInput")
    bias_d = nc.dram_tensor("bias", [KU, P, 1], F32, kind="ExternalInput")
    eye_d = nc.dram_tensor("eye", [P, P], DT, kind="ExternalInput")
    out_d = nc.dram_tensor("out", [P, T * KU * B_c], DT, kind="ExternalOutput")

    with tile.TileContext(nc) as tc:
        with (
            tc.tile_pool(name="consts", bufs=1) as cpool,
            tc.tile_pool(name="dram", bufs=1, space="DRAM") as dpool,
        ):
            # resident weights: wk_sb/wr_sb [128, KD*U] with col = k*U + u
            wk_sb = cpool.tile([P, KD * U], DT, tag="wk", name="wk")
            wr_sb = cpool.tile([P, KU * U], DT, tag="wr", name="wr")
            bias_sb = cpool.tile([P, KU], F32, tag="bias", name="bias")
            for k in range(KD):
                nc.sync.dma_start(
                    wk_sb[:, k * U : (k + 1) * U], wk_d[k * P : (k + 1) * P, :]
                )
            for k in range(KU):
                nc.sync.dma_start(
                    wr_sb[:, k * U : (k + 1) * U], wr_d[k * P : (k + 1) * P, :]
                )
            for m in range(KU):
                nc.sync.dma_start(bias_sb[:, m : m + 1], bias_d[m])

            xin_dram = dpool.tile([U, NCOL], DT, tag="xin", name="xin")

            # ---------------- Phase 1: input GEMM ----------------
            with (
                tc.tile_pool(name="g_xt", bufs=3) as xt_pool,
                tc.tile_pool(name="g_ps", bufs=8, space="PSUM") as gps_pool,
                tc.tile_pool(name="g_ev", bufs=3) as ev_pool,
            ):
                for j in range(NCOL // GN):
                    xts = []
                    for k in range(KD):
                        xt = xt_pool.tile([P, GN], DT, tag=f"xt{k}", name=f"xt{k}")
                        nc.sync.dma_start(
                            xt[:],
                            xT_d[k * P : (k + 1) * P, j * GN : (j + 1) * GN],
                        )
                        xts.append(xt)
                    for m in range(KU):
                        ps = gps_pool.tile([P, GN], F32, tag="gps", name="gps")
                        for k in range(KD):
                            nc.tensor.matmul(
                                ps[:],
                                wk_sb[:, k * U + m * P : k * U + (m + 1) * P],
                                xts[k][:],
                                start=(k == 0),
                                stop=(k == KD - 1),
                            )
                        ev = ev_pool.tile([P, GN], DT, tag="ev", name="ev")
                        nc.scalar.activation(
                            ev[:], ps[:], AF.Identity, bias=bias_sb[:, m : m + 1]
                        )
                        nc.sync.dma_start(
                            xin_dram[m * P : (m + 1) * P, j * GN : (j + 1) * GN],
                            ev[:],
                        )

            # ---------------- Phase 2: recurrence ----------------
            # The host supplies wr already scaled by LEAKY (0.9) and rescales
            # the output by 0.9, so the on-device recurrence is
            #   hh_t = 0.1*hh_{t-1} + tanh(u_t + hh_{t-1} @ (0.9 R))
            # with h = 0.9*hh. Per step: one identity matmul streams u_t into
            # PSUM (start=True), 16 rec matmuls accumulate, one ACT tanh, one
            # fused DVE op hh = 0.1*hp + th. PSUM col order n = m*B_c + b;
            # out-block o is t-major (cols t*NB + n) so every per-step DVE/ACT
            # AP is contiguous; the state slices the rec matmuls stream are
            # contiguous [128, B_c] as well.
            NB = KU * B_c  # columns per step (32)
            with (
                tc.tile_pool(name="r_u", bufs=1) as u_pool,
                tc.tile_pool(name="r_o", bufs=1) as o_pool,
                tc.tile_pool(name="r_ps", bufs=2, space="PSUM") as rps_pool,
                tc.tile_pool(name="r_tmp", bufs=2) as tmp_pool,
                tc.tile_pool(name="r_state", bufs=1) as st_pool,
            ):
                eye_sb = cpool.tile([P, P], DT, tag="eye", name="eye")
                nc.sync.dma_start(eye_sb[:], eye_d.ap())
                h_state = st_pool.tile([P, NB], DT, tag="hs", name="hs")
                nc.vector.memset(h_state[:], 0.0)

                xin_v = [
                    xin_dram[m * P : (m + 1) * P, :].rearrange(
                        "p (b t) -> p b t", t=T
                    )
                    for m in range(KU)
                ]

                def body(i):
                    # u block: m-major contiguous (DMA-friendly); the identity
                    # matmul streams it with a strided AP.
                    u_blk = u_pool.tile([P, NB * T_blk], DT, tag="u", name="u")
                    for m in range(KU):
                        dst = u_blk[
                            :, m * B_c * T_blk : (m + 1) * B_c * T_blk
                        ].rearrange("p (b t) -> p b t", t=T_blk)
                        nc.sync.dma_start(dst, xin_v[m][:, :, bass.ds(i, T_blk)])
                    u_mbt = u_blk.rearrange(
                        "p (m b t) -> p m b t", m=KU, b=B_c
                    )
                    # o block: t-major
                    o_blk = o_pool.tile([P, T_blk * NB], DT, tag="o", name="o")
                    o_t = o_blk.rearrange("p (t n) -> p t n", n=NB)

                    for t in range(T_blk):
                        ps = rps_pool.tile([P, NB], F32, tag="ps", name="ps")
                        # u_t -> PSUM via identity matmul (clears the bank)
                        nc.tensor.matmul(
                            ps[:],
                            eye_sb[:],
                            u_mbt[:, :, :, t],
                            start=True,
                            stop=False,
                            skip_group_check=True,
                        )
                        hp = h_state[:] if t == 0 else o_t[:, t - 1, :]
                        for m in range(KU):
                            for k in range(KU):
                                nc.tensor.matmul(
                                    ps[:, m * B_c : (m + 1) * B_c],
                                    wr_sb[:, k * U + m * P : k * U + (m + 1) * P],
                                    hp[:, k * B_c : (k + 1) * B_c],
                                    start=False,
                                    stop=(m == KU - 1 and k == KU - 1),
                                    skip_group_check=True,
                                )
                        th = tmp_pool.tile([P, NB], F32, tag="th", name="th")
                        nc.scalar.activation(th[:], ps[:], AF.Tanh)
                        # hh_t = 0.1*hp + th
                        nc.vector.scalar_tensor_tensor(
                            o_t[:, t, :], hp, 0.1, th[:], Alu.mult, Alu.add
                        )

                    nc.vector.tensor_copy(h_state[:], o_t[:, T_blk - 1, :])
                    # dump the block verbatim; host unshuffles [p, t, m, b]
                    nc.sync.dma_start(
                        out_d[:, bass.ds(i * NB, T_blk * NB)], o_blk[:]
                    )

                import concourse.mybir as _mybir

                with tc.For_i(
                    0,
                    T,
                    T_blk,
                    hint_engines=(
                        _mybir.EngineType.PE,
                        _mybir.EngineType.DVE,
                        _mybir.EngineType.Activation,
                    ),
                ) as i:
                    body(i)

    _split_multi_waits(nc)
    return nc


_PROG_CACHE = {}


def _get_program(T, B_c, D, U):
    key = (T, B_c, D, U)
    if key not in _PROG_CACHE:
        _apply_ntff_shim()
        _PROG_CACHE[key] = _build_program(T, B_c, D, U)
    return _PROG_CACHE[key]


def _run(x, kernel_f, rec_f, bias_f, kernel_b, rec_b, bias_b, trace=False):
    from concourse.bass_utils import run_bass_kernel_spmd

    x = np.asarray(x)
    B, T, D = x.shape
    U = np.asarray(kernel_f).shape[1]
    n_grp = N_CORES // 2
    assert B % n_grp == 0
    B_c = B // n_grp

    nc = _get_program(T, B_c, D, U)

    wks = [np.asarray(kernel_f), np.asarray(kernel_b)]
    wrs = [np.asarray(rec_f), np.asarray(rec_b)]
    biases = [np.asarray(bias_f), np.asarray(bias_b)]

    eye = np.eye(P, dtype=np.float16)
    in_maps = []
    metas = []
    for c in range(N_CORES):
        d, g = c // n_grp, c % n_grp
        xs = x[g * B_c : (g + 1) * B_c]
        if d == 1:
            xs = xs[:, ::-1, :]
        xT = np.ascontiguousarray(
            xs.transpose(2, 0, 1).reshape(D, B_c * T)
        ).astype(np.float16)
        in_maps.append(
            {
                "xt": xT,
                "wk": wks[d].astype(np.float16),
                # device recurrence runs on hh = h/0.9 with wr' = 0.9*wr
                "wr": (LEAKY * wrs[d]).astype(np.float16),
                "bias": biases[d].reshape(U // P, P, 1).astype(np.float32),
                "eye": eye,
            }
        )
        metas.append((d, g))

    res = run_bass_kernel_spmd(nc, in_maps, list(range(N_CORES)), trace=trace)

    out = np.empty((B, T, 2 * U), dtype=np.float32)
    for c in range(N_CORES):
        d, g = metas[c]
        # device layout: out[p, t*KU*B_c + m*B_c + b] = hh[m*128+p, b, t]
        o = res.results[c]["out"].astype(np.float32)
        o = o.reshape(P, T, U // P, B_c).transpose(3, 1, 2, 0)
        o = o.reshape(B_c, T, U) * np.float32(LEAKY)
        if d == 1:
            o = o[:, ::-1, :]
        out[g * B_c : (g + 1) * B_c, :, d * U : (d + 1) * U] = o
    return out, res


def kernel(x, kernel_f, rec_f, bias_f, kernel_b, rec_b, bias_b):
    out, _ = _run(x, kernel_f, rec_f, bias_f, kernel_b, rec_b, bias_b)
    return out
